# revision 17
# baseline (speedup 1.0000x reference)
"""Trainium2 Bass kernel for nn_CellLineMLPPredictor.

Computation (B=512 samples):
  x0 = concat(h_drug[pairs[:,0]], attrs[:,1:2], h_drug[pairs[:,1]], attrs[:,3:4])  [B, 2048]
  x1 = relu(x0 @ W0.T + b0)      [B, 2048]
  x2 = relu(x1 @ W1.T + b1)      [B, 1024]
  z  = relu(einsum('boi,bi->bo', L0[cl], x2) + O0[cl,:,0])  [B, 512]
  y  = einsum('boi,bi->bo', L1[cl], z) + O1[cl,:,0]          [B, 1] -> [B]

Strategy (8 cores, no collectives):
  - Host routing: samples sorted by cell line. Core c owns cell lines
    [4c, 4c+4); its samples are packed into 4 groups of G padded columns
    (G = max group count rounded to 8). All per-sample gathers (h_drug,
    L1, O0, O1 selection) become dense per-group matmuls.
  - All activations are kept feature-major ("transposed": [features,
    samples]), so every layer is out.T = W @ x.T and the natural [out,
    in] weight layout transposed once on host gives lhsT tiles directly.
  - The kernel is DMA-bound streaming replicated weights, so W0 and W1
    are stored as scaled fp8 e3m4 and fed to the PE directly as the
    stationary operand (mixed fp8 lhsT x fp16 rhs matmul works and the
    HW result matches the numpy e3m4 simulation exactly); the dequant
    scales fold into the stage-1/2 activation epilogues for free.
    L0 stays fp16 (expert-sharded, read exactly once machine-wide).
    End-to-end rel err on the fixed inputs: 1.51e-2 vs the 2e-2 gate.
  - Everything streams on the two HWDGE rings (no SWDGE: its descriptor
    refills serialize onto DMA engine 0). Sync ring: w0p, w1p, then l0p
    so the last-arriving chunk only gates stage-3 group 3's small tail.
    Scalar ring: x0 + packed consts + y. Weight packs are
    [chunk, 128, 4096], one fully-contiguous DMA each.
"""

import numpy as np


try:
    import concourse.bass  # noqa: F401
except ImportError:  # grading environment may not have it on sys.path
    import sys

    for _p in ("/opt/trn_rl_repo", "/root/.axon_site/_ro/trn_rl_repo"):
        if _p not in sys.path:
            sys.path.insert(0, _p)

B = 512
N_CELL = 32
N_CORE = 8
GROUPS_PER_CORE = N_CELL // N_CORE  # 4
D_IN = 2048
P = 128  # partitions

FP8_MAX = 15.0  # e3m4 max finite is 15.5; leave rounding headroom

LAST_RUN = None  # BassKernelResults of the most recent kernel() call
_PROG_CACHE = {}  # (G, s0, s1) -> compiled Bass program


def _get_program(G, s0, s1):
    key = (G, float(s0), float(s1))
    if key not in _PROG_CACHE:
        _PROG_CACHE[key] = _build_program(G, float(s0), float(s1))
    return _PROG_CACHE[key]


def _build_program(G, s0, s1):
    """Build the SPMD Bass program. G = padded per-group column count.
    s0/s1 = fp8 dequant scales for W0/W1."""
    import concourse.bacc as bacc
    import concourse.mybir as mybir
    from concourse.tile import TileContext

    f32 = mybir.dt.float32
    f16 = mybir.dt.float16
    f8e3 = mybir.dt.float8e3
    Relu = mybir.ActivationFunctionType.Relu
    Identity = mybir.ActivationFunctionType.Identity
    Add = mybir.AluOpType.add
    Max = mybir.AluOpType.max

    NCOL = GROUPS_PER_CORE * G  # columns (samples) per core

    nc = bacc.Bacc("TRN2", target_bir_lowering=False)

    # Per-core inputs (pre-packed on host into SBUF-ready layouts).
    # Weight packs are [n_chunks, 128, chunk_cols]: each chunk is several
    # contraction tiles side by side in the free dim, one contiguous DMA.
    # x0 is split in two so stage-1 can start after the first half lands.
    x0a = nc.dram_tensor("x0a", [P, 8 * NCOL], f16, kind="ExternalInput")
    x0b = nc.dram_tensor("x0b", [P, 8 * NCOL], f16, kind="ExternalInput")
    w0p = nc.dram_tensor("w0p", [8, P, 4096], f8e3, kind="ExternalInput")
    w1p = nc.dram_tensor("w1p", [4, P, 4096], f8e3, kind="ExternalInput")
    l0p = nc.dram_tensor("l0p", [8, P, 2048], f16, kind="ExternalInput")
    # packed f32 consts: b0 (16) | b1 (8) | o0 (16) -> [128, 40]
    cm = nc.dram_tensor("cm", [P, 40], f32, kind="ExternalInput")
    l1m = nc.dram_tensor("l1m", [P, 16], f16, kind="ExternalInput")
    o1m = nc.dram_tensor("o1m", [1, 4], f32, kind="ExternalInput")
    y = nc.dram_tensor("y", [1, NCOL], f32, kind="ExternalOutput")

    with TileContext(nc) as tc:
        with (
            tc.tile_pool(name="consts", bufs=1) as consts,
            tc.tile_pool(name="acts", bufs=1) as acts,
            tc.tile_pool(name="w0pool", bufs=8) as w0pool,
            tc.tile_pool(name="w1pool", bufs=4) as w1pool,
            tc.tile_pool(name="l0pool", bufs=8) as l0pool,
            tc.tile_pool(name="psum", bufs=8, space="PSUM") as psum,
        ):
            # small/early loads on the scalar HWDGE ring; the sync ring
            # carries nothing but the weight stream
            x0asb = acts.tile([P, 8 * NCOL], f16, tag="x0a")
            nc.scalar.dma_start(x0asb[:], x0a[:])
            x0bsb = acts.tile([P, 8 * NCOL], f16, tag="x0b")
            nc.scalar.dma_start(x0bsb[:], x0b[:])
            csb = consts.tile([P, 40], f32, tag="csb")
            nc.scalar.dma_start(csb[:], cm[:])
            l1sb = consts.tile([P, 16], f16, tag="l1sb")
            nc.scalar.dma_start(l1sb[:], l1m[:])
            o1sb = consts.tile([1, 4], f32, tag="o1sb")
            nc.scalar.dma_start(o1sb[:], o1m[:])
            b0sb = csb[:, 0:16]
            b1sb = csb[:, 16:24]
            o0sb = csb[:, 24:40]

            x1sb = acts.tile([P, 16 * NCOL], f16, tag="x1sb")
            x2sb = acts.tile([P, 8 * NCOL], f16, tag="x2sb")
            zsb = acts.tile([P, 16 * G], f16, tag="zsb")
            ysb = acts.tile([1, NCOL], f32, tag="ysb")

            # ---- stage 1: x1.T = relu(s0 * (W0q @ x0.T) + b0), M=2048 in 2 halves
            for mh in range(2):
                ps = [
                    psum.tile([P, NCOL], f32, tag="ps", name=f"ps{i}")
                    for i in range(8)
                ]
                for c in range(4):
                    wt = w0pool.tile([P, 4096], f8e3, tag="w0", name="w0t")
                    nc.sync.dma_start(wt[:], w0p[mh * 4 + c])
                    for kk in range(4):
                        k = c * 4 + kk
                        xs = x0asb if k < 8 else x0bsb
                        for mi in range(8):
                            nc.tensor.matmul(
                                ps[mi][:],
                                wt[:, kk * 1024 + mi * 128 : kk * 1024 + (mi + 1) * 128],
                                xs[:, (k % 8) * NCOL : (k % 8 + 1) * NCOL],
                                start=(k == 0),
                                stop=(k == 15),
                            )
                for mi in range(8):
                    m = mh * 8 + mi
                    nc.scalar.activation(
                        x1sb[:, m * NCOL : (m + 1) * NCOL],
                        ps[mi][:],
                        Relu,
                        bias=b0sb[:, m : m + 1],
                        scale=s0,
                    )

            # ---- stage 2: x2.T = relu(s1 * (W1q @ x1.T) + b1), M=1024
            ps2 = [
                psum.tile([P, NCOL], f32, tag="ps", name=f"ps{i}") for i in range(8)
            ]
            for c in range(4):
                wt = w1pool.tile([P, 4096], f8e3, tag="w1", name="w1t")
                nc.sync.dma_start(wt[:], w1p[c])
                for kk in range(4):
                    k = c * 4 + kk
                    for mi in range(8):
                        nc.tensor.matmul(
                            ps2[mi][:],
                            wt[:, kk * 1024 + mi * 128 : kk * 1024 + (mi + 1) * 128],
                            x1sb[:, k * NCOL : (k + 1) * NCOL],
                            start=(k == 0),
                            stop=(k == 15),
                        )
            for mi in range(8):
                nc.scalar.activation(
                    x2sb[:, mi * NCOL : (mi + 1) * NCOL],
                    ps2[mi][:],
                    Relu,
                    bias=b1sb[:, mi : mi + 1],
                    scale=s1,
                )

            # ---- stage 3 + 4, interleaved so stage-4 of group g overlaps
            # stage-3 of group g+2 on the in-order PE queue.
            # stage 3: z_g.T = relu(L0[c_g] @ x2_g.T + O0); l0p[g] holds
            # L0[c_g].T as 8 k-tiles of [128, 512]. These ride the sync ring
            # AFTER the shared weights, so the last-arriving chunk (l0p[3])
            # only gates group 3's small matmuls.
            # stage 4: y_g = L1[c_g] @ z_g.T + O1 -> [1, G]
            lts = []
            for h in range(8):
                lt = l0pool.tile([P, 2048], f16, tag="l0", name=f"lt{h}")
                nc.sync.dma_start(lt[:], l0p[h])
                lts.append(lt)

            def stage3(g):
                ps3 = [
                    psum.tile([P, G], f32, tag="ps", name=f"ps3_{i}")
                    for i in range(4)
                ]
                for k in range(8):
                    wt = lts[2 * g + k // 4]
                    for mi in range(4):
                        nc.tensor.matmul(
                            ps3[mi][:],
                            wt[:, (k % 4) * 512 + mi * 128 : (k % 4) * 512 + (mi + 1) * 128],
                            x2sb[:, k * NCOL + g * G : k * NCOL + (g + 1) * G],
                            start=(k == 0),
                            stop=(k == 7),
                        )
                for mi in range(4):
                    # fused (psum + O0) then max(0) on the otherwise-idle DVE
                    nc.vector.tensor_scalar(
                        zsb[:, (g * 4 + mi) * G : (g * 4 + mi + 1) * G],
                        ps3[mi][:],
                        o0sb[:, g * 4 + mi : g * 4 + mi + 1],
                        0.0,
                        Add,
                        Max,
                    )

            def stage4(g):
                ps4 = psum.tile([1, G], f32, tag="ps", name="ps4")
                for k in range(4):
                    nc.tensor.matmul(
                        ps4[:],
                        l1sb[:, g * 4 + k : g * 4 + k + 1],
                        zsb[:, (g * 4 + k) * G : (g * 4 + k + 1) * G],
                        start=(k == 0),
                        stop=(k == 3),
                    )
                nc.scalar.activation(
                    ysb[0:1, g * G : (g + 1) * G],
                    ps4[0:1, :],
                    Identity,
                    bias=o1sb[0:1, g : g + 1],
                )
                # per-group output DMA on the (by now idle) sync ring, so
                # only group 3's sliver trails the last compute
                nc.sync.dma_start(
                    y[0:1, g * G : (g + 1) * G], ysb[0:1, g * G : (g + 1) * G]
                )

            stage3(0)
            stage3(1)
            stage4(0)
            stage3(2)
            stage4(1)
            stage3(3)
            stage4(2)
            stage4(3)

    nc.compile()
    return nc


def kernel(**inputs):
    global LAST_RUN
    import os

    import ml_dtypes

    from concourse.bass_utils import run_bass_kernel_spmd

    pairs = np.asarray(inputs["pairs"]).astype(np.int64)
    cell_lines = np.asarray(inputs["cell_lines"]).astype(np.int64)
    attrs = np.asarray(inputs["attrs"], dtype=np.float32)
    h_drug = np.asarray(inputs["h_drug"], dtype=np.float32)
    W0 = np.asarray(inputs["W0"], dtype=np.float32)
    b0 = np.asarray(inputs["b0"], dtype=np.float32)
    W1 = np.asarray(inputs["W1"], dtype=np.float32)
    b1 = np.asarray(inputs["b1"], dtype=np.float32)
    L0 = np.asarray(inputs["L0"], dtype=np.float32)
    O0 = np.asarray(inputs["O0"], dtype=np.float32)
    L1 = np.asarray(inputs["L1"], dtype=np.float32)
    O1 = np.asarray(inputs["O1"], dtype=np.float32)

    n_attr = attrs.shape[1] // 2
    # x0.T, feature-major: [2048, B]
    x0T = np.empty((D_IN, B), dtype=np.float32)
    x0T[:1023] = h_drug[pairs[:, 0]].T
    x0T[1023] = attrs[:, n_attr - 1]
    x0T[1024:2047] = h_drug[pairs[:, 1]].T
    x0T[2047] = attrs[:, -1]

    counts = np.bincount(cell_lines, minlength=N_CELL)
    G = max(8, int(-(-counts.max() // 8) * 8))
    NCOL = GROUPS_PER_CORE * G
    # one PSUM bank per [128, NCOL] f32 accumulator; 8 live at once
    assert NCOL <= 512, f"group padding {G} too large for single-bank PSUM tiles"
    groups = [np.where(cell_lines == c)[0] for c in range(N_CELL)]

    # shared (replicated) weight packs as scaled fp8 e3m4, chunk-of-4-
    # ktiles layout; dequant scales fold into the activation epilogues.
    s0 = float(np.abs(W0).max()) / FP8_MAX
    w0q = (W0 / s0).astype(ml_dtypes.float8_e3m4)
    w0k = w0q.reshape(2, 1024, 16, P).transpose(0, 2, 3, 1).reshape(32, P, 1024)
    w0p = np.ascontiguousarray(
        w0k.reshape(2, 4, 4, P, 1024).transpose(0, 1, 3, 2, 4).reshape(8, P, 4096)
    ).view(np.int8)
    s1 = float(np.abs(W1).max()) / FP8_MAX
    w1q = (W1 / s1).astype(ml_dtypes.float8_e3m4)
    w1k = w1q.T.reshape(16, P, 1024)
    w1p = np.ascontiguousarray(
        w1k.reshape(4, 4, P, 1024).transpose(0, 2, 1, 3).reshape(4, P, 4096)
    ).view(np.int8)
    b0m = b0.reshape(16, P).T
    b1m = b1.reshape(8, P).T

    in_maps = []
    for core in range(N_CORE):
        cells = [GROUPS_PER_CORE * core + i for i in range(GROUPS_PER_CORE)]
        x0c = np.zeros((D_IN, NCOL), dtype=np.float32)
        for gi, c in enumerate(cells):
            idx = groups[c]
            x0c[:, gi * G : gi * G + len(idx)] = x0T[:, idx]
        x0k = x0c.reshape(16, P, NCOL).astype(np.float16)
        x0a = np.ascontiguousarray(
            x0k[:8].transpose(1, 0, 2).reshape(P, 8 * NCOL)
        )
        x0b = np.ascontiguousarray(
            x0k[8:].transpose(1, 0, 2).reshape(P, 8 * NCOL)
        )
        # l0p[2g + h] = L0[c_g].T k-tiles [4h:4h+4] as [128, 4*512]
        l0p = np.ascontiguousarray(
            np.stack(
                [
                    L0[c].T.reshape(2, 4, P, 512)[h].transpose(1, 0, 2).reshape(P, 2048)
                    for c in cells
                    for h in range(2)
                ]
            )
        ).astype(np.float16)
        o0m = (
            np.stack([O0[c][:, 0].reshape(4, P) for c in cells])
            .transpose(2, 0, 1)
            .reshape(P, 16)
        )
        cmix = np.ascontiguousarray(
            np.concatenate([b0m, b1m, o0m], axis=1).astype(np.float32)
        )
        l1m = np.ascontiguousarray(
            np.stack([L1[c][0].reshape(4, P) for c in cells])
            .transpose(2, 0, 1)
            .reshape(P, 16)
        ).astype(np.float16)
        o1m = np.ascontiguousarray(
            np.array([[O1[c, 0, 0] for c in cells]], dtype=np.float32)
        )
        in_maps.append(
            {
                "x0a": x0a,
                "x0b": x0b,
                "w0p": w0p,
                "w1p": w1p,
                "l0p": l0p,
                "cm": cmix,
                "l1m": l1m,
                "o1m": o1m,
            }
        )

    nc = _get_program(G, s0, s1)
    trace = bool(os.environ.get("BENCH_TRACE"))
    LAST_RUN = run_bass_kernel_spmd(nc, in_maps, list(range(N_CORE)), trace=trace)
    results = LAST_RUN.results

    out = np.zeros(B, dtype=np.float32)
    for core in range(N_CORE):
        yc = results[core]["y"]
        for gi in range(GROUPS_PER_CORE):
            c = GROUPS_PER_CORE * core + gi
            idx = groups[c]
            out[idx] = yc[0, gi * G : gi * G + len(idx)]
    return out
